# revision 10
# baseline (speedup 1.0000x reference)
"""Trainium2 Bass kernel for nn_Graph_to_Featuremaps_savemem.

Math: the reference computes, per batch b,
    scores[b,p,n] = (res @ nfr)[b,p] + (x @ nfh)[b,n]
    attn = softmax_n(scores);  out[b,p,c] = (attn @ (x @ W))[b,p,c]
Softmax over n is shift-invariant, so the (res @ nfr)[b,p] term cancels:
    attn[b,p,:] = softmax(x[b] @ nfh)   (independent of p)
    out[b,c,h,w] = relu(((softmax(x[b]@nfh) @ x[b]) @ W)[c])   broadcast over (h,w)
res_feature never affects the output. The kernel is a tiny per-batch compute
(one 64-softmax + two small matmuls) followed by a 256 MB broadcast write --
pure HBM-write-bound, sharded batch-parallel over 8 cores (2 batches, 32 MB
written per core).

Device-side chain (inputs cast to bf16 on host; X passed pre-transposed so
there is no PE transpose; all matmuls are single-pass bf16 with fp32 PSUM):
  s  = X @ nfh                 (128,1)  one matmul (lhsT = XT)
  e  = exp(s)                  (128,1)  bf16 out
  M  = X @ W                   (128,256) one matmul, copied to SBUF as bf16
  S_b = ONES[b-rows]^T @ e[b]  (128,1)  per-batch sum broadcast to all parts
  RC[:,b] = 1/S_b              (128,2)
  V[b,c] = M[b-rows,c]^T @ e[b] (128,1) per (batch, c-half)
  fill[b,c] = (0 max V) * RC[:,b]  broadcast to (128, 2048) on DVE
Output: per (batch, c-half) row-block, 8 x 1 MB DMAs (128 partitions x 8 KB
descriptors at 64 KB dest stride -- measured at the per-packet floor, ~419
GB/s) alternating the two HWDGE rings (SP / ACT).

Straggler mitigation: DMA work is split across the 16 SDMA engines statically
by source partition; SDMA engine 15 (serving partitions 92-95/124-127) is
~19% slower than the rest on about half of the runs, which makes it the sole
tail of the whole stream (+15 us). We spill rows 92/93 of blocks 0/1 and row
92 of blocks 2/3 (48 of engine 15's 256 packets) to partitions owned by fast
engines: a host-provided 0/1 permutation matrix moves the (relu'd, scaled)
row values across partitions on the PE, and 6 small extra DMAs write those
rows from there. Main DMAs exclude the spilled rows.
"""

import numpy as np

N_CORES = 8
B, NODES, HID, C, H, W = 16, 64, 128, 256, 128, 128
HWP = H * W  # 16384
B_LOC = B // N_CORES  # 2 batches per core
BN = B_LOC * NODES  # 128
FILL_F = 2048  # free-dim width of the broadcast fill tiles in SBUF
NBLK = 4  # output row-blocks per core: (batch, c-half)

# spill plan: block j spills rows SPILL_ROWS[j] (all owned by SDMA engine 15)
# to spill-source partition 8*j + 4*i (engines 0,2,4,...  all fast)
SPILL_ROWS = [(92, 93), (92, 93), (92,), (92,)]


def _spill_q(j, i):
    return 8 * j + 4 * i


_NC_CACHE = {}


def build_nc():
    import concourse.bass as bass
    import concourse.bacc as bacc
    import concourse.mybir as mybir
    from concourse.tile import TileContext

    f32 = mybir.dt.float32
    bf16 = mybir.dt.bfloat16
    Alu = mybir.AluOpType
    Act = mybir.ActivationFunctionType

    nc = bacc.Bacc(None, target_bir_lowering=False, debug=False)
    # merged input A: col 0 = nfh, cols 1:129 = X^T, cols 129:385 = W
    # (one DMA, 770 B/partition contiguous -- tiny-descriptor loads of the
    # separate (128,1)/(128,128) tensors poison the ring with 2 B descriptors)
    inp_d = nc.declare_dram_parameter("inp", [HID, 1 + BN + C], bf16, isOutput=False)
    # merged input B: four 128x128 0/1 spill permutation matrices
    pmat_d = nc.declare_dram_parameter("pmat", [128, NBLK * 128], bf16, isOutput=False)
    out_d = nc.declare_dram_parameter("out", [B_LOC * C, HWP], f32, isOutput=True)

    with TileContext(nc) as tc:
        with (
            tc.tile_pool(name="singles", bufs=1) as singles,
            tc.tile_pool(name="psum", bufs=1, space="PSUM") as psum,
        ):
            # ---- inputs first so their DMAs issue as early as possible ----
            INP = singles.tile([HID, 1 + BN + C], bf16, tag="INP")
            nc.sync.dma_start(out=INP[:], in_=inp_d[:])
            PM = singles.tile([128, NBLK * 128], bf16, tag="PM")
            nc.sync.dma_start(out=PM[:], in_=pmat_d[:])
            NFH = INP[:, 0:1]
            XT = INP[:, 1 : 1 + BN]
            Wt = INP[:, 1 + BN : 1 + BN + C]

            # ---- constants (no input deps) ----
            ONES128 = singles.tile([128, 128], bf16, tag="ONES128")
            nc.vector.memset(ONES128[:], 1.0)
            ZERO = singles.tile([128, FILL_F], f32, tag="ZERO")
            nc.vector.memset(ZERO[:], 0.0)

            # ---- s = X @ nfh ; e = exp(s) (bf16) ----
            s_ps = psum.tile([BN, 1], f32, tag="s")
            nc.tensor.matmul(s_ps[:], XT, NFH)
            e_col = singles.tile([BN, 1], bf16, tag="e_col")
            nc.scalar.activation(e_col[:], s_ps[:], Act.Exp)

            # ---- M = X @ W (independent of the e-chain) ----
            M_ps = psum.tile([BN, C], f32, tag="M")
            nc.tensor.matmul(M_ps[:], XT, Wt)
            M_sb = singles.tile([BN, C], bf16, tag="M_sb")
            nc.vector.tensor_copy(M_sb[:], M_ps[:])

            # ---- per-batch sums broadcast to all partitions; RC = 1/S ----
            RC = singles.tile([128, B_LOC], f32, tag="RC")
            S_ps = []
            for b in range(B_LOC):
                sl = slice(b * NODES, (b + 1) * NODES)
                sp = psum.tile([128, 1], f32, tag=f"S{b}")
                nc.tensor.matmul(sp[:], ONES128[sl, :], e_col[sl, :])
                S_ps.append(sp)
            for b in range(B_LOC):
                nc.vector.reciprocal(RC[:, b : b + 1], S_ps[b][:])

            # ---- V[b,c] = M[b-rows, c-half]^T @ e[b] : (128,1) each ----
            # then fill = (0 max V) * RC[:,b] and the main output DMAs,
            # with engine-15's spilled rows excluded from the main DMAs
            RV = singles.tile([128, NBLK], bf16, tag="RV")  # relu'd cols for spill
            V_ps = []
            for j in range(NBLK):
                b, hf = divmod(j, C // 128)
                sl = slice(b * NODES, (b + 1) * NODES)
                vp = psum.tile([128, 1], f32, tag=f"V{j}")
                nc.tensor.matmul(
                    vp[:], M_sb[sl, hf * 128 : (hf + 1) * 128], e_col[sl, :]
                )
                V_ps.append(vp)
                fill = singles.tile([128, FILL_F], f32, tag=f"fill{j}")
                nc.vector.tensor_scalar(
                    fill[:], ZERO[:], vp[:], RC[:, b : b + 1], op0=Alu.max, op1=Alu.mult
                )
                r0 = j * 128
                lo, hi = SPILL_ROWS[j][0], SPILL_ROWS[j][-1] + 1
                for k in range(HWP // FILL_F):
                    eng = nc.sync if k % 2 == 0 else nc.scalar
                    cols = slice(k * FILL_F, (k + 1) * FILL_F)
                    eng.dma_start(
                        out=out_d[r0 : r0 + lo, cols], in_=fill[0:lo, :]
                    )
                    eng.dma_start(
                        out=out_d[r0 + hi : r0 + 128, cols], in_=fill[hi:128, :]
                    )

            # ---- spill: move the excluded rows' values to fast partitions
            # via the 0/1 permutation matmuls, then 6 small row DMAs ----
            for j in range(NBLK):
                b = j // (C // 128)
                nc.vector.tensor_scalar(
                    RV[:, j : j + 1],
                    V_ps[j][:],
                    RC[:, b : b + 1],
                    0.0,
                    op0=Alu.mult,
                    op1=Alu.max,
                )
            SPv = psum.tile([128, 1], f32, tag="s")  # reuse s's bank (s is dead)
            for j in range(NBLK):
                nc.tensor.matmul(
                    SPv[:],
                    PM[:, j * 128 : (j + 1) * 128],
                    RV[:, j : j + 1],
                    start=(j == 0),
                    stop=(j == NBLK - 1),
                )
            SPfill = singles.tile([128, FILL_F], f32, tag="SPfill")
            nc.vector.tensor_scalar(SPfill[:], ZERO[:], SPv[:], None, op0=Alu.add)
            nspill = 0
            for j in range(NBLK):
                for i, row in enumerate(SPILL_ROWS[j]):
                    q = _spill_q(j, i)
                    src = SPfill[q : q + 1, :]
                    src_b = type(src)(
                        src.tensor,
                        src.offset,
                        [list(src.ap[0]), [0, HWP // FILL_F], list(src.ap[1])],
                    )
                    eng = nc.sync if nspill % 2 == 0 else nc.scalar
                    nspill += 1
                    eng.dma_start(
                        out=out_d[j * 128 + row : j * 128 + row + 1, :], in_=src_b
                    )
    nc.finalize()
    return nc


def get_nc():
    if "nc" not in _NC_CACHE:
        _NC_CACHE["nc"] = build_nc()
    return _NC_CACHE["nc"]


def _make_pmat():
    pm = np.zeros((128, NBLK * 128), dtype=np.float32)
    for j in range(NBLK):
        for i, row in enumerate(SPILL_ROWS[j]):
            pm[row, j * 128 + _spill_q(j, i)] = 1.0
    return pm


def make_in_maps(input, node_fea_for_hidden, weight):
    import ml_dtypes

    bf16 = ml_dtypes.bfloat16
    x = np.asarray(input, np.float32)[0]  # (B, NODES, HID)
    nfh = np.asarray(node_fea_for_hidden, np.float32).reshape(HID, 1)
    w = np.asarray(weight, np.float32)
    pmat = np.ascontiguousarray(_make_pmat().astype(bf16))
    in_maps = []
    for i in range(N_CORES):
        xt = x[i * B_LOC : (i + 1) * B_LOC].reshape(BN, HID).T
        inp = np.concatenate([nfh, xt, w], axis=1).astype(bf16)
        in_maps.append({"inp": np.ascontiguousarray(inp), "pmat": pmat})
    return in_maps


def run_spmd(in_maps, trace=False, **kw):
    from concourse.bass_utils import run_bass_kernel_spmd

    return run_bass_kernel_spmd(get_nc(), in_maps, list(range(N_CORES)), trace=trace, **kw)


def kernel(input, res_feature, node_fea_for_res, node_fea_for_hidden, weight):
    res = run_spmd(make_in_maps(input, node_fea_for_hidden, weight)).results
    out = np.concatenate(
        [r["out"].reshape(B_LOC, C, H, W) for r in res], axis=0
    )
    return out


# revision 11
# speedup vs baseline: 3.7095x; 3.7095x over previous
"""Trainium2 Bass kernel for nn_Graph_to_Featuremaps_savemem.

Math: the reference computes, per batch b,
    scores[b,p,n] = (res @ nfr)[b,p] + (x @ nfh)[b,n]
    attn = softmax_n(scores);  out[b,p,c] = (attn @ (x @ W))[b,p,c]
Softmax over n is shift-invariant, so the (res @ nfr)[b,p] term cancels:
    attn[b,p,:] = softmax(x[b] @ nfh)   (independent of p)
    out[b,c,h,w] = relu(((softmax(x[b]@nfh) @ x[b]) @ W)[c])   broadcast over (h,w)
res_feature never affects the output. The kernel is a tiny per-batch compute
(one 64-softmax + two small matmuls) followed by a 256 MB broadcast write --
pure HBM-write-bound, sharded batch-parallel over 8 cores (2 batches, 32 MB
written per core).

Device-side chain (inputs cast to bf16 on host, merged into one (128,385)
tensor so the load is a single DMA with 770 B/partition descriptors; X is
passed pre-transposed so there is no PE transpose; all matmuls are
single-pass bf16 with fp32 PSUM accumulation):
  s  = X @ nfh                 (128,1)  one matmul (lhsT = XT)
  e  = exp(s)                  (128,1)  bf16 out
  M  = X @ W                   (128,256) one matmul, copied to SBUF as bf16
  S_b = ONES[b-rows]^T @ e[b]  (128,1)  per-batch sum broadcast to all parts
  RC[:,b] = 1/S_b              (128,2)
  V[b,c] = M[b-rows,c]^T @ e[b] (128,1) per (batch, c-half)
  fill[b,c] = (0 max V) * RC[:,b]  broadcast to (128, 2048) on DVE
Output: per (batch, c-half) row-block, 8 x 1 MB DMAs (128 partitions x 8 KB
descriptors at 64 KB dest stride) alternating the two HWDGE rings (SP/ACT).
This shape measures at the per-packet floor (~315 ns / 8 KB descriptor,
~419 GB/s aggregate).  Full 128-partition DMAs are essential: partial
partition ranges collapse the descriptor spread onto a few SDMA engines
(measured 4x slowdown), and stride-0 source APs or contiguous-dest variants
measure ~14% slower per packet.
"""

import numpy as np

N_CORES = 8
B, NODES, HID, C, H, W = 16, 64, 128, 256, 128, 128
HWP = H * W  # 16384
B_LOC = B // N_CORES  # 2 batches per core
BN = B_LOC * NODES  # 128
FILL_F = 2048  # free-dim width of the broadcast fill tiles in SBUF
NBLK = 4  # output row-blocks per core: (batch, c-half)

_NC_CACHE = {}


def build_nc():
    import concourse.bass as bass
    import concourse.bacc as bacc
    import concourse.mybir as mybir
    from concourse.tile import TileContext

    f32 = mybir.dt.float32
    bf16 = mybir.dt.bfloat16
    Alu = mybir.AluOpType
    Act = mybir.ActivationFunctionType

    nc = bacc.Bacc(None, target_bir_lowering=False, debug=False)
    # merged input: col 0 = nfh, cols 1:129 = X^T, cols 129:385 = W
    inp_d = nc.declare_dram_parameter("inp", [HID, 1 + BN + C], bf16, isOutput=False)
    out_d = nc.declare_dram_parameter("out", [B_LOC * C, HWP], f32, isOutput=True)

    with TileContext(nc) as tc:
        with (
            tc.tile_pool(name="singles", bufs=1) as singles,
            tc.tile_pool(name="psum", bufs=1, space="PSUM") as psum,
        ):
            # ---- input first so its DMA issues as early as possible ----
            INP = singles.tile([HID, 1 + BN + C], bf16, tag="INP")
            nc.sync.dma_start(out=INP[:], in_=inp_d[:])
            NFH = INP[:, 0:1]
            XT = INP[:, 1 : 1 + BN]
            Wt = INP[:, 1 + BN : 1 + BN + C]

            # ---- constants (no input deps) ----
            ONES128 = singles.tile([128, 128], bf16, tag="ONES128")
            nc.vector.memset(ONES128[:], 1.0)
            ZERO = singles.tile([128, FILL_F], f32, tag="ZERO")
            nc.vector.memset(ZERO[:], 0.0)

            # ---- s = X @ nfh ; e = exp(s) (bf16) ----
            s_ps = psum.tile([BN, 1], f32, tag="s")
            nc.tensor.matmul(s_ps[:], XT, NFH)
            e_col = singles.tile([BN, 1], bf16, tag="e_col")
            nc.scalar.activation(e_col[:], s_ps[:], Act.Exp)

            # ---- M = X @ W (independent of the e-chain) ----
            M_ps = psum.tile([BN, C], f32, tag="M")
            nc.tensor.matmul(M_ps[:], XT, Wt)
            M_sb = singles.tile([BN, C], bf16, tag="M_sb")
            nc.vector.tensor_copy(M_sb[:], M_ps[:])

            # ---- per-batch sums broadcast to all partitions; RC = 1/S ----
            RC = singles.tile([128, B_LOC], f32, tag="RC")
            S_ps = []
            for b in range(B_LOC):
                sl = slice(b * NODES, (b + 1) * NODES)
                sp = psum.tile([128, 1], f32, tag=f"S{b}")
                nc.tensor.matmul(sp[:], ONES128[sl, :], e_col[sl, :])
                S_ps.append(sp)
            for b in range(B_LOC):
                nc.vector.reciprocal(RC[:, b : b + 1], S_ps[b][:])

            # ---- V[b,c] = M[b-rows, c-half]^T @ e[b] : (128,1) each,
            # fill = (0 max V) * RC[:,b], then 8 x 1 MB DMAs per block ----
            for j in range(NBLK):
                b, hf = divmod(j, C // 128)
                sl = slice(b * NODES, (b + 1) * NODES)
                vp = psum.tile([128, 1], f32, tag=f"V{j}")
                nc.tensor.matmul(
                    vp[:], M_sb[sl, hf * 128 : (hf + 1) * 128], e_col[sl, :]
                )
                fill = singles.tile([128, FILL_F], f32, tag=f"fill{j}")
                nc.vector.tensor_scalar(
                    fill[:], ZERO[:], vp[:], RC[:, b : b + 1], op0=Alu.max, op1=Alu.mult
                )
                r0 = j * 128
                for k in range(HWP // FILL_F):
                    eng = nc.sync if k % 2 == 0 else nc.scalar
                    eng.dma_start(
                        out=out_d[r0 : r0 + 128, k * FILL_F : (k + 1) * FILL_F],
                        in_=fill[:],
                    )
    nc.finalize()
    return nc


def get_nc():
    if "nc" not in _NC_CACHE:
        _NC_CACHE["nc"] = build_nc()
    return _NC_CACHE["nc"]


def make_in_maps(input, node_fea_for_hidden, weight):
    import ml_dtypes

    bf16 = ml_dtypes.bfloat16
    x = np.asarray(input, np.float32)[0]  # (B, NODES, HID)
    nfh = np.asarray(node_fea_for_hidden, np.float32).reshape(HID, 1)
    w = np.asarray(weight, np.float32)
    in_maps = []
    for i in range(N_CORES):
        xt = x[i * B_LOC : (i + 1) * B_LOC].reshape(BN, HID).T
        inp = np.concatenate([nfh, xt, w], axis=1).astype(bf16)
        in_maps.append({"inp": np.ascontiguousarray(inp)})
    return in_maps


def run_spmd(in_maps, trace=False, **kw):
    from concourse.bass_utils import run_bass_kernel_spmd

    return run_bass_kernel_spmd(get_nc(), in_maps, list(range(N_CORES)), trace=trace, **kw)


def kernel(input, res_feature, node_fea_for_res, node_fea_for_hidden, weight):
    res = run_spmd(make_in_maps(input, node_fea_for_hidden, weight)).results
    out = np.concatenate(
        [r["out"].reshape(B_LOC, C, H, W) for r in res], axis=0
    )
    return out
